# revision 15
# baseline (speedup 1.0000x reference)
"""Swin-style window-attention transformer block on 8 Trainium2 NeuronCores.

Data-parallel over batch B=8 (one image per core). Inside each core:
  - tokens kept window-ordered: band = one row of 8 windows (512 tokens),
    4 groups of 128 tokens per band, each group = one pair of windows
    (partitions 0:64 = even window of the pair, 64:128 = odd window).
  - LayerNorm stats via DVE bn_stats token-major; rstd via the
    magic-constant Newton rsqrt on DVE (no ACT Ln -> single activation
    table set for the whole kernel); the per-token scale/shift applied
    alternately on ACT and DVE so the four groups finish in two serial op
    times; LN affine folded into the following GEMM weights on the host.
  - activations transposed to feature-major with batched xbar DMA
    transposes ([128,512] -> [128,4,128] per 128-token group), alternating
    between the sync and scalar HWDGE rings to halve the serial latency.
  - window attention in bf16: QK^T row-packed 4 heads per PE pass, one
    PSUM bank per head (concurrent row-packed matmuls must drain into
    distinct banks; only the first 256 fp32 of each bank are used),
    rel-pos bias added on DVE, exp on ACT into a persistent
    block-diagonal score buffer; softmax denominators via ones-matmuls
    inverted with the DVE fast reciprocal; AV col-packed 4 heads into one
    PSUM bank, normalization folded into the PSUM->SBUF evacuation
    multiply.
"""

import functools
import numpy as np

B, H, W, C = 8, 64, 64, 512
WH, WW = 8, 8
NH = 16
HD = C // NH
EPS = 1e-5
P = 128
NB = 8          # bands per core (window rows)
NG = 4          # 128-token groups per band (window pairs)
TB = 512        # tokens per band
N_CORES = 8

RSQRT_MAGIC = 0x5F3759DF


def _rel_pos_index():
    coords = np.stack(np.meshgrid(np.arange(WH), np.arange(WW), indexing="ij"))
    cf = coords.reshape(2, -1)
    rel = (cf[:, :, None] - cf[:, None, :]).transpose(1, 2, 0).copy()
    rel[..., 0] += WH - 1
    rel[..., 1] += WW - 1
    rel[..., 0] *= 2 * WW - 1
    return rel.sum(-1)  # [64, 64] int


def _emit(nc, tc, ctx, aps):
    import concourse.mybir as mybir

    dt = mybir.dt
    f32, bf16, u32 = dt.float32, dt.bfloat16, dt.uint32
    AF = mybir.ActivationFunctionType
    ALU = mybir.AluOpType

    x_r = aps["x"].rearrange(
        "(wr i) (wcp wcl j) c -> wr wcp wcl i j c", i=8, wcl=2, j=8)
    out_r = aps["out"].rearrange(
        "(wr i) (wcp wcl j) c -> wr wcp wcl i j c", i=8, wcl=2, j=8)

    wqkv_r = aps["wqkv"].rearrange("(cc p) f -> p cc f", p=P)    # [128,4,1536]
    wproj_r = aps["wproj"].rearrange("(cc p) f -> p cc f", p=P)  # [128,4,512]
    w1_r = aps["w1"].rearrange("(cc p) (fc f) -> p cc fc f", p=P, f=P)
    w2_r = aps["w2"].rearrange("(fc p) f -> p fc f", p=P)        # [128,16,512]

    const = ctx.enter_context(tc.tile_pool(name="const", bufs=1))
    xpool = ctx.enter_context(tc.tile_pool(name="xp", bufs=2))
    hpool = ctx.enter_context(tc.tile_pool(name="hp", bufs=2))
    hTpool = ctx.enter_context(tc.tile_pool(name="hTp", bufs=2))
    qkpool = ctx.enter_context(tc.tile_pool(name="qkp", bufs=2))
    vpool = ctx.enter_context(tc.tile_pool(name="vp", bufs=2))
    opool = ctx.enter_context(tc.tile_pool(name="op", bufs=2))
    m1pool = ctx.enter_context(tc.tile_pool(name="m1p", bufs=2))
    outpool = ctx.enter_context(tc.tile_pool(name="outp", bufs=2))
    spool = ctx.enter_context(tc.tile_pool(name="sp", bufs=2))
    rdpool = ctx.enter_context(tc.tile_pool(name="rdp", bufs=2))

    # qkv/attention and proj/MLP use separate PSUM pools so next-band qkv
    # isn't slot-chained behind this band's MLP; the attention pd/po tiles
    # ride the qkv ring (same slot size, transient per-cc lifetime)
    pp_qkv = ctx.enter_context(tc.tile_pool(name="ppqkv", bufs=2, space="PSUM"))
    pp_mlp = ctx.enter_context(tc.tile_pool(name="ppmlp", bufs=2, space="PSUM"))
    pp_s = ctx.enter_context(tc.tile_pool(name="pps", bufs=1, space="PSUM"))

    # ---- persistent constants (weight loads split across the scalar
    # HWDGE ring and the gpsimd SWDGE ring so startup isn't serialized on
    # one DMA path; qkv weights first - they gate the first band) ----
    wqkv_sb = const.tile([P, 4, 3 * C], bf16)
    nc.scalar.dma_start(wqkv_sb[:, :, :2 * C], wqkv_r[:, :, :2 * C])
    nc.gpsimd.dma_start(wqkv_sb[:, :, 2 * C:], wqkv_r[:, :, 2 * C:])
    wproj_sb = const.tile([P, 4, C], bf16)
    nc.gpsimd.dma_start(wproj_sb[:], wproj_r)
    w1_sb = const.tile([P, 4, 16, P], bf16)
    nc.scalar.dma_start(w1_sb[:], w1_r)
    w2_sb = const.tile([P, 16, C], bf16)
    nc.gpsimd.dma_start(w2_sb[:], w2_r)
    bqkc_sb = const.tile([P, 8], f32)       # q,k bias columns (1024 feats)
    nc.gpsimd.dma_start(bqkc_sb[:], aps["bqkc"])
    bm1c_sb = const.tile([P, 16], f32)
    nc.gpsimd.dma_start(bm1c_sb[:], aps["bm1c"])
    bvbc_sb = const.tile([P, C], bf16)
    nc.gpsimd.dma_start(bvbc_sb[:], aps["bvbc"])
    bpbc_sb = const.tile([P, C], bf16)
    nc.gpsimd.dma_start(bpbc_sb[:], aps["bpbc"])
    bm2bc_sb = const.tile([P, C], bf16)
    nc.gpsimd.dma_start(bm2bc_sb[:], aps["bm2bc"])
    biasT_sb = const.tile([P, NH, 64], bf16)
    nc.gpsimd.dma_start(biasT_sb[:], aps["biasT"])
    ones_sb = const.tile([P, 32], bf16)
    nc.vector.memset(ones_sb[:], 1.0)
    # persistent block-diagonal score buffer [key(128), head, pair, wcl, 64]
    scores_sb = const.tile([P, NH, NG, 2, 64], bf16)
    nc.vector.memset(scores_sb[:], 0.0)

    def load_stats(band):
        x_sb = xpool.tile([P, NG, C], f32, tag="x")
        st = spool.tile([P, NG, 6], f32, tag="st1", name="st1")
        mv = spool.tile([P, NG, 2], f32, tag="mv1", name="mv1")
        for g in range(NG):
            for wcl in range(2):
                nc.sync.dma_start(x_sb[wcl * 64:(wcl + 1) * 64, g],
                                  x_r[band, g, wcl])
            nc.vector.bn_stats(out=st[:, g], in_=x_sb[:, g])
            nc.vector.bn_aggr(out=mv[:, g], in_=st[:, g])
        return x_sb, mv

    def ln_rstd(mv, nm):
        # a = rsqrt(var+eps), b = -mean*a on DVE (magic-constant Newton,
        # no ACT table-set switch)
        a_t = spool.tile([P, NG], f32, tag=f"a{nm}", name=f"a{nm}")
        b_t = spool.tile([P, NG], f32, tag=f"b{nm}", name=f"b{nm}")
        t_t = spool.tile([P, NG], f32, tag=f"t{nm}", name=f"t{nm}")
        nc.vector.tensor_scalar_add(b_t[:], mv[:, :, 1], EPS)   # ve = var+eps
        yu, vu = a_t[:].bitcast(u32), b_t[:].bitcast(u32)
        nc.vector.tensor_scalar(yu, vu, 1, None, ALU.logical_shift_right)
        nc.vector.tensor_scalar(yu, yu, RSQRT_MAGIC ^ 0xFFFFFFFF, None, ALU.add)
        nc.vector.tensor_scalar(yu, yu, 0xFFFFFFFF, None, ALU.bitwise_xor)
        for _ in range(1):  # Newton: a *= 1.5 - 0.5*ve*a^2 (~0.1% rstd err)
            nc.vector.tensor_tensor(t_t[:], a_t[:], a_t[:], ALU.mult)
            nc.vector.tensor_tensor(t_t[:], t_t[:], b_t[:], ALU.mult)
            nc.vector.tensor_scalar(t_t[:], t_t[:], -0.5, 1.5, ALU.mult, ALU.add)
            nc.vector.tensor_tensor(a_t[:], a_t[:], t_t[:], ALU.mult)
        nc.vector.tensor_tensor(b_t[:], mv[:, :, 0], a_t[:], ALU.mult)
        nc.vector.tensor_scalar_mul(b_t[:], b_t[:], -1.0)
        return a_t, b_t

    def ln_apply_transpose(x_sb, ab, htag):
        # apply alternating ACT/DVE (parallel engines halve the stage),
        # then batched xbar transpose per group on the sync ring
        a_t, b_t = ab
        h_sb = hpool.tile([P, NG, C], bf16, tag="h")
        hT_sb = hTpool.tile([P, 4, TB], bf16, tag=htag)
        for g in range(NG):
            if g % 2 == 0:
                nc.scalar.activation(h_sb[:, g], x_sb[:, g], AF.Identity,
                                     bias=b_t[:, g:g + 1],
                                     scale=a_t[:, g:g + 1])
            else:
                nc.vector.tensor_scalar(
                    h_sb[:, g], x_sb[:, g], a_t[:, g:g + 1], b_t[:, g:g + 1],
                    ALU.mult, ALU.add)
            nc.sync.dma_start(hT_sb[:, :, g * P:(g + 1) * P], h_sb[:, g],
                              transpose=True)
        return hT_sb

    def attn_cc(cc, qk_sb, v_sb, oT_sb):
        # QK^T for 4 heads, one PSUM bank per head-tile i (concurrent
        # row-packed matmuls must drain into distinct banks); only the
        # first 256 fp32 of each bank are used.
        pss = pp_s.tile([P, 4, TB], f32, tag="pss")
        psv = pss[:, :, 0:NG * 64].rearrange("p i (g q) -> p i g q", g=NG)
        for w in range(8):
            pairi, wcl = divmod(w, 2)
            for i in range(4):
                out_sl = (psv[0:64, i, pairi] if wcl == 0
                          else psv[64:128, i, pairi])
                nc.tensor.matmul(
                    out_sl,
                    qk_sb[32 * i:32 * (i + 1), 1, cc, w * 64:(w + 1) * 64],
                    qk_sb[32 * i:32 * (i + 1), 0, cc, w * 64:(w + 1) * 64],
                    start=True, stop=True,
                    tile_position=(32 * i, 0 if wcl == 0 else 64))
        nc.vector.tensor_tensor(
            psv[:], psv[:],
            biasT_sb[:, 4 * cc:4 * cc + 4, None, :].to_broadcast(
                (P, 4, NG, 64)),
            ALU.add)
        nc.scalar.activation(scores_sb[0:64, 4 * cc:4 * cc + 4, :, 0, :],
                             psv[0:64], AF.Exp)
        nc.scalar.activation(scores_sb[64:128, 4 * cc:4 * cc + 4, :, 1, :],
                             psv[64:128], AF.Exp)
        # softmax denominators: col-packed ones-matmuls + fast reciprocal
        pd = pp_qkv.tile([P, NG, P], f32, tag="big")
        for pair in range(NG):
            for j in range(4):
                nc.tensor.matmul(
                    pd[32 * j:32 * (j + 1), pair], ones_sb[:],
                    scores_sb[:, 4 * cc + j, pair],
                    start=True, stop=True, tile_position=(0, 32 * j))
        rd = rdpool.tile([P, NG, P], f32, tag="rd")
        nc.vector.reciprocal_approx_fast(rd[:], pd[:])
        # AV col-packed 4 heads into one PSUM bank; normalize on the
        # PSUM->SBUF evacuation multiply
        po = pp_qkv.tile([P, NG, P], f32, tag="big")
        for pair in range(NG):
            for j in range(4):
                nc.tensor.matmul(
                    po[32 * j:32 * (j + 1), pair],
                    v_sb[:, pair, (4 * cc + j) * HD:(4 * cc + j + 1) * HD],
                    scores_sb[:, 4 * cc + j, pair],
                    start=True, stop=True, tile_position=(0, 32 * j))
        nc.vector.tensor_tensor(oT_sb[:, cc], po[:], rd[:], ALU.mult)

    # ---- software pipeline: band b+1's LN1 pieces are emitted at points
    # where their engines (DVE stats/rstd, ACT+DVE applies, sync
    # transposes) idle during band b's attention, so hT(b+1) is ready when
    # band b's LN2 chain stalls the PE and next-band qkv can fill the gap
    x_sb, mv1 = load_stats(0)
    hT_sb = ln_apply_transpose(x_sb, ln_rstd(mv1, "1"), "hT")
    for band in range(NB):
        # ---- QKV: q,k feature-major bf16 ----
        qk_sb = qkpool.tile([P, 2, 4, TB], bf16, tag="qk")
        for f in range(8):
            ps = pp_qkv.tile([P, TB], f32, tag="big")
            for cc in range(4):
                nc.tensor.matmul(
                    ps[:], wqkv_sb[:, cc, f * P:(f + 1) * P],
                    hT_sb[:, cc],
                    start=(cc == 0), stop=(cc == 3))
            qi, ci = divmod(f, 4)
            nc.any.tensor_scalar_add(qk_sb[:, qi, ci], ps[:],
                                     bqkc_sb[:, f:f + 1])

        # ---- QKV: v token-major bf16 ----
        v_sb = vpool.tile([P, NG, C], bf16, tag="v")
        for g in range(NG):
            ps = pp_qkv.tile([P, C], f32, tag="big")
            for cc in range(4):
                nc.tensor.matmul(
                    ps[:], hT_sb[:, cc, g * P:(g + 1) * P],
                    wqkv_sb[:, cc, 2 * C:3 * C],
                    start=(cc == 0), stop=(cc == 3))
            nc.any.tensor_tensor(v_sb[:, g], ps[:], bvbc_sb[:], ALU.add)

        if band + 1 < NB:
            nxt_x, nxt_mv = load_stats(band + 1)

        # ---- window attention (bf16) ----
        oT_sb = opool.tile([P, 4, TB], bf16, tag="oT")
        attn_cc(0, qk_sb, v_sb, oT_sb)
        attn_cc(1, qk_sb, v_sb, oT_sb)
        if band + 1 < NB:
            nxt_ab = ln_rstd(nxt_mv, "1")
        attn_cc(2, qk_sb, v_sb, oT_sb)
        attn_cc(3, qk_sb, v_sb, oT_sb)
        if band + 1 < NB:
            nxt_hT = ln_apply_transpose(nxt_x, nxt_ab, "hT")

        # ---- proj + residual -> y (overwrites x tile) ----
        st2 = spool.tile([P, NG, 6], f32, tag="st2", name="st2")
        mv2 = spool.tile([P, NG, 2], f32, tag="mv2", name="mv2")
        for g in range(NG):
            ps = pp_mlp.tile([P, C], f32, tag="mlp")
            for cc in range(4):
                nc.tensor.matmul(
                    ps[:], oT_sb[:, cc, g * P:(g + 1) * P],
                    wproj_sb[:, cc],
                    start=(cc == 0), stop=(cc == 3))
            nc.any.tensor_tensor(x_sb[:, g], ps[:], x_sb[:, g], ALU.add)
            nc.any.tensor_tensor(x_sb[:, g], x_sb[:, g], bpbc_sb[:], ALU.add)
            nc.vector.bn_stats(out=st2[:, g], in_=x_sb[:, g])
            nc.vector.bn_aggr(out=mv2[:, g], in_=st2[:, g])
        y_sb = x_sb

        # ---- LN2 + transpose ----
        h2T_sb = ln_apply_transpose(y_sb, ln_rstd(mv2, "2"), "h2T")

        # ---- MLP + final residual ----
        m1_sb = m1pool.tile([P, 16, TB], bf16, tag="m1")
        for fc in range(16):
            ps = pp_mlp.tile([P, TB], f32, tag="mlp")
            for cc in range(4):
                nc.tensor.matmul(
                    ps[:], w1_sb[:, cc, fc], h2T_sb[:, cc],
                    start=(cc == 0), stop=(cc == 3))
            nc.scalar.activation(m1_sb[:, fc], ps[:], AF.Relu,
                                 bias=bm1c_sb[:, fc:fc + 1])
        for g in range(NG):
            ps = pp_mlp.tile([P, C], f32, tag="mlp")
            for fc in range(16):
                nc.tensor.matmul(
                    ps[:], m1_sb[:, fc, g * P:(g + 1) * P], w2_sb[:, fc],
                    start=(fc == 0), stop=(fc == 15))
            o_sb = outpool.tile([P, C], f32, tag="out")
            nc.any.tensor_tensor(o_sb[:], ps[:], y_sb[:, g], ALU.add)
            nc.any.tensor_tensor(o_sb[:], o_sb[:], bm2bc_sb[:], ALU.add)
            for wcl in range(2):
                nc.gpsimd.dma_start(out_r[band, g, wcl],
                                    o_sb[wcl * 64:(wcl + 1) * 64])

        if band + 1 < NB:
            x_sb, hT_sb = nxt_x, nxt_hT


@functools.lru_cache(maxsize=2)
def _build():
    from contextlib import ExitStack
    import concourse.mybir as mybir
    import concourse.tile as tile
    from concourse import bacc

    dt = mybir.dt
    nc = bacc.Bacc("TRN2", target_bir_lowering=False, debug=False,
                   num_devices=N_CORES)
    aps = {}
    specs = [
        ("x", [H, W, C], dt.float32),
        ("wqkv", [C, 3 * C], dt.bfloat16),
        ("wproj", [C, C], dt.bfloat16),
        ("w1", [C, 4 * C], dt.bfloat16),
        ("w2", [4 * C, C], dt.bfloat16),
        ("bqkc", [P, 8], dt.float32),
        ("bm1c", [P, 16], dt.float32),
        ("bvbc", [P, C], dt.bfloat16),
        ("bpbc", [P, C], dt.bfloat16),
        ("bm2bc", [P, C], dt.bfloat16),
        ("biasT", [P, NH, 64], dt.bfloat16),
    ]
    for name, shape, dtype in specs:
        aps[name] = nc.dram_tensor(name, shape, dtype,
                                   kind="ExternalInput").ap()
    aps["out"] = nc.dram_tensor("out", [H, W, C], dt.float32,
                                kind="ExternalOutput").ap()
    with tile.TileContext(nc) as tc:
        with ExitStack() as ctx:
            _emit(nc, tc, ctx, aps)
    nc.compile()
    return nc


def _prepare_in_maps(x, g1, b1, wqkv, bqkv, wproj, bproj, rel_bias, g2, b2,
                     w1, bm1, w2, bm2):
    x = np.asarray(x, np.float32)
    f = lambda a: np.ascontiguousarray(np.asarray(a, np.float32))
    g1, b1, wqkv, bqkv = f(g1), f(b1), f(wqkv), f(bqkv)
    wproj, bproj, rel_bias = f(wproj), f(bproj), f(rel_bias)
    g2, b2, w1, bm1, w2, bm2 = f(g2), f(b2), f(w1), f(bm1), f(w2), f(bm2)

    # fold LN1 affine into wqkv/bqkv; fold attention scale into q weights
    wqkv_f = g1[:, None] * wqkv
    bqkv_f = b1 @ wqkv + bqkv
    sc = HD ** -0.5
    wqkv_f[:, :C] *= sc
    bqkv_f[:C] *= sc
    # fold LN2 affine into w1/bm1
    w1_f = g2[:, None] * w1
    bm1_f = b2 @ w1 + bm1

    bqkc = np.ascontiguousarray(bqkv_f[:2 * C].reshape(8, P).T)   # [128, 8]
    bm1c = np.ascontiguousarray(bm1_f.reshape(16, P).T)           # [128, 16]
    import ml_dtypes
    bfarr = lambda a: np.ascontiguousarray(a).astype(ml_dtypes.bfloat16)
    bvbc = bfarr(np.broadcast_to(bqkv_f[2 * C:], (P, C)))
    bpbc = bfarr(np.broadcast_to(bproj, (P, C)))
    bm2bc = bfarr(np.broadcast_to(bm2, (P, C)))

    idx = _rel_pos_index()                              # [64(n), 64(m)]
    bias_nm = rel_bias[idx, :]                          # [n, m, NH]
    biasT_h = bias_nm.transpose(2, 1, 0)                # [NH, m, n]
    biasT = np.concatenate([biasT_h, biasT_h], axis=1)  # [NH, 128, 64]
    biasT = bfarr(biasT.transpose(1, 0, 2))             # [128, NH, 64]

    wqkv_b, wproj_b, w1_b, w2_b = (bfarr(wqkv_f), bfarr(wproj),
                                   bfarr(w1_f), bfarr(w2))
    shared = dict(wqkv=wqkv_b, wproj=wproj_b, w1=w1_b, w2=w2_b,
                  bqkc=bqkc, bm1c=bm1c, bvbc=bvbc, bpbc=bpbc, bm2bc=bm2bc,
                  biasT=biasT)
    return [dict(x=np.ascontiguousarray(x[c]), **shared)
            for c in range(N_CORES)]


def kernel(**inputs):
    from concourse.bass_utils import run_bass_kernel_spmd

    in_maps = _prepare_in_maps(**inputs)
    nc = _build()
    res = run_bass_kernel_spmd(nc, in_maps, core_ids=list(range(N_CORES)))
    return np.stack([res.results[c]["out"] for c in range(N_CORES)], axis=0)


# revision 18
# speedup vs baseline: 1.4638x; 1.4638x over previous
"""Swin-style window-attention transformer block on 8 Trainium2 NeuronCores.

Data-parallel over batch B=8 (one image per core). Inside each core, a
2-band-deep software pipeline (the Tile scheduler keeps per-engine FIFO
order close to emission order, so the overlap is hand-interleaved at
instruction granularity):

  - attention(b) interleaves mlp1(b-1) matmul groups right after each
    head-quarter's exp is issued, so the PE computes MLP work during the
    softmax chain's ACT/DVE stages instead of stalling on the score
    buffer.
  - proj(b) interleaves mlp2(b-1) + output stores group-by-group.
  - the LN2(b) and LN1(b+1) chains (bn_stats, Newton-rsqrt on DVE,
    scale/shift applies alternating ACT/DVE, batched xbar transposes on
    sync) run back-to-back while qkv(b+1) fills the PE.
  - x band loads and output stores ride the gpsimd SWDGE ring; the sync
    HWDGE ring carries only the transposes.

Layout/math identical to the flat version: tokens window-ordered per band
(512 tokens = 4 groups = 4 window pairs), LN stats token-major via
bn_stats, rstd via magic-constant Newton rsqrt (no ACT Ln -> single
activation table set), LN affine folded into GEMM weights host-side,
window attention in bf16 with row-packed QK^T (one PSUM bank per packed
head), rel-pos bias on DVE, exp into a persistent block-diagonal score
buffer, softmax denominators via ones-matmuls + DVE fast reciprocal, AV
col-packed with normalization folded into the PSUM evacuation multiply.
"""

import functools
import numpy as np

B, H, W, C = 8, 64, 64, 512
WH, WW = 8, 8
NH = 16
HD = C // NH
EPS = 1e-5
P = 128
NB = 8          # bands per core (window rows)
NG = 4          # 128-token groups per band (window pairs)
TB = 512        # tokens per band
N_CORES = 8

RSQRT_MAGIC = 0x5F3759DF


def _rel_pos_index():
    coords = np.stack(np.meshgrid(np.arange(WH), np.arange(WW), indexing="ij"))
    cf = coords.reshape(2, -1)
    rel = (cf[:, :, None] - cf[:, None, :]).transpose(1, 2, 0).copy()
    rel[..., 0] += WH - 1
    rel[..., 1] += WW - 1
    rel[..., 0] *= 2 * WW - 1
    return rel.sum(-1)  # [64, 64] int


def _emit(nc, tc, ctx, aps):
    import concourse.mybir as mybir

    dt = mybir.dt
    f32, bf16, u32 = dt.float32, dt.bfloat16, dt.uint32
    AF = mybir.ActivationFunctionType
    ALU = mybir.AluOpType

    x_r = aps["x"].rearrange(
        "(wr i) (wcp wcl j) c -> wr wcp wcl i j c", i=8, wcl=2, j=8)
    out_r = aps["out"].rearrange(
        "(wr i) (wcp wcl j) c -> wr wcp wcl i j c", i=8, wcl=2, j=8)

    wqkv_r = aps["wqkv"].rearrange("(cc p) f -> p cc f", p=P)    # [128,4,1536]
    wproj_r = aps["wproj"].rearrange("(cc p) f -> p cc f", p=P)  # [128,4,512]
    w1_r = aps["w1"].rearrange("(cc p) (fc f) -> p cc fc f", p=P, f=P)
    w2_r = aps["w2"].rearrange("(fc p) f -> p fc f", p=P)        # [128,16,512]

    const = ctx.enter_context(tc.tile_pool(name="const", bufs=1))
    xpool = ctx.enter_context(tc.tile_pool(name="xp", bufs=3))
    hpool = ctx.enter_context(tc.tile_pool(name="hp", bufs=2))
    hTpool = ctx.enter_context(tc.tile_pool(name="hTp", bufs=2))
    qkpool = ctx.enter_context(tc.tile_pool(name="qkp", bufs=2))
    vpool = ctx.enter_context(tc.tile_pool(name="vp", bufs=2))
    opool = ctx.enter_context(tc.tile_pool(name="op", bufs=2))
    m1pool = ctx.enter_context(tc.tile_pool(name="m1p", bufs=2))
    outpool = ctx.enter_context(tc.tile_pool(name="outp", bufs=2))
    spool = ctx.enter_context(tc.tile_pool(name="sp", bufs=2))
    rdpool = ctx.enter_context(tc.tile_pool(name="rdp", bufs=2))

    # qkv/attention vs proj/MLP on separate PSUM pools so the interleaved
    # streams aren't slot-chained through each other; the attention pd/po
    # tiles ride the qkv ring (same slot size, transient per-cc lifetime)
    pp_qkv = ctx.enter_context(tc.tile_pool(name="ppqkv", bufs=2, space="PSUM"))
    pp_mlp = ctx.enter_context(tc.tile_pool(name="ppmlp", bufs=2, space="PSUM"))
    pp_s = ctx.enter_context(tc.tile_pool(name="pps", bufs=1, space="PSUM"))

    # ---- persistent constants (weight loads split across the scalar
    # HWDGE ring and the gpsimd SWDGE ring; qkv weights first - they gate
    # the first band) ----
    wqkv_sb = const.tile([P, 4, 3 * C], bf16)
    nc.scalar.dma_start(wqkv_sb[:, :, :2 * C], wqkv_r[:, :, :2 * C])
    nc.gpsimd.dma_start(wqkv_sb[:, :, 2 * C:], wqkv_r[:, :, 2 * C:])
    wproj_sb = const.tile([P, 4, C], bf16)
    nc.gpsimd.dma_start(wproj_sb[:], wproj_r)
    w1_sb = const.tile([P, 4, 16, P], bf16)
    nc.scalar.dma_start(w1_sb[:], w1_r)
    w2_sb = const.tile([P, 16, C], bf16)
    nc.gpsimd.dma_start(w2_sb[:], w2_r)
    bqkc_sb = const.tile([P, 8], f32)       # q,k bias columns (1024 feats)
    nc.gpsimd.dma_start(bqkc_sb[:], aps["bqkc"])
    bm1c_sb = const.tile([P, 16], f32)
    nc.gpsimd.dma_start(bm1c_sb[:], aps["bm1c"])
    bvbc_sb = const.tile([P, C], bf16)
    nc.gpsimd.dma_start(bvbc_sb[:], aps["bvbc"])
    bpbc_sb = const.tile([P, C], bf16)
    nc.gpsimd.dma_start(bpbc_sb[:], aps["bpbc"])
    bm2bc_sb = const.tile([P, C], bf16)
    nc.gpsimd.dma_start(bm2bc_sb[:], aps["bm2bc"])
    biasT_sb = const.tile([P, NH, 64], bf16)
    nc.gpsimd.dma_start(biasT_sb[:], aps["biasT"])
    ones_sb = const.tile([P, 32], bf16)
    nc.vector.memset(ones_sb[:], 1.0)
    # persistent block-diagonal score buffer [key(128), head, pair, wcl, 64]
    scores_sb = const.tile([P, NH, NG, 2, 64], bf16)
    nc.vector.memset(scores_sb[:], 0.0)

    def load_stats(band):
        x_sb = xpool.tile([P, NG, C], f32, tag="x")
        st = spool.tile([P, NG, 6], f32, tag="st1", name="st1")
        mv = spool.tile([P, NG, 2], f32, tag="mv1", name="mv1")
        for g in range(NG):
            for wcl in range(2):
                nc.gpsimd.dma_start(x_sb[wcl * 64:(wcl + 1) * 64, g],
                                    x_r[band, g, wcl])
            nc.vector.bn_stats(out=st[:, g], in_=x_sb[:, g])
            nc.vector.bn_aggr(out=mv[:, g], in_=st[:, g])
        return x_sb, mv

    def ln_rstd(mv, nm):
        # a = rsqrt(var+eps), b = -mean*a on DVE (magic-constant Newton,
        # no ACT table-set switch)
        a_t = spool.tile([P, NG], f32, tag=f"a{nm}", name=f"a{nm}")
        b_t = spool.tile([P, NG], f32, tag=f"b{nm}", name=f"b{nm}")
        t_t = spool.tile([P, NG], f32, tag=f"t{nm}", name=f"t{nm}")
        nc.vector.tensor_scalar_add(b_t[:], mv[:, :, 1], EPS)   # ve = var+eps
        yu, vu = a_t[:].bitcast(u32), b_t[:].bitcast(u32)
        nc.vector.tensor_scalar(yu, vu, 1, None, ALU.logical_shift_right)
        nc.vector.tensor_scalar(yu, yu, RSQRT_MAGIC ^ 0xFFFFFFFF, None, ALU.add)
        nc.vector.tensor_scalar(yu, yu, 0xFFFFFFFF, None, ALU.bitwise_xor)
        for _ in range(1):  # Newton: a *= 1.5 - 0.5*ve*a^2 (~0.1% rstd err)
            nc.vector.tensor_tensor(t_t[:], a_t[:], a_t[:], ALU.mult)
            nc.vector.tensor_tensor(t_t[:], t_t[:], b_t[:], ALU.mult)
            nc.vector.tensor_scalar(t_t[:], t_t[:], -0.5, 1.5, ALU.mult, ALU.add)
            nc.vector.tensor_tensor(a_t[:], a_t[:], t_t[:], ALU.mult)
        nc.vector.tensor_tensor(b_t[:], mv[:, :, 0], a_t[:], ALU.mult)
        nc.vector.tensor_scalar_mul(b_t[:], b_t[:], -1.0)
        return a_t, b_t

    def ln_apply_transpose(x_sb, ab, htag):
        # apply alternating ACT/DVE (parallel engines halve the stage),
        # then batched xbar transpose per group on the sync ring
        a_t, b_t = ab
        h_sb = hpool.tile([P, NG, C], bf16, tag="h")
        hT_sb = hTpool.tile([P, 4, TB], bf16, tag=htag)
        for g in range(NG):
            if g % 2 == 0:
                nc.scalar.activation(h_sb[:, g], x_sb[:, g], AF.Identity,
                                     bias=b_t[:, g:g + 1],
                                     scale=a_t[:, g:g + 1])
            else:
                nc.vector.tensor_scalar(
                    h_sb[:, g], x_sb[:, g], a_t[:, g:g + 1], b_t[:, g:g + 1],
                    ALU.mult, ALU.add)
            nc.sync.dma_start(hT_sb[:, :, g * P:(g + 1) * P], h_sb[:, g],
                              transpose=True)
        return hT_sb

    def emit_qkv(hT_sb):
        qk_sb = qkpool.tile([P, 2, 4, TB], bf16, tag="qk")
        for f in range(8):
            ps = pp_qkv.tile([P, TB], f32, tag="big")
            for cc in range(4):
                nc.tensor.matmul(
                    ps[:], wqkv_sb[:, cc, f * P:(f + 1) * P],
                    hT_sb[:, cc],
                    start=(cc == 0), stop=(cc == 3))
            qi, ci = divmod(f, 4)
            nc.any.tensor_scalar_add(qk_sb[:, qi, ci], ps[:],
                                     bqkc_sb[:, f:f + 1])
        v_sb = vpool.tile([P, NG, C], bf16, tag="v")
        for g in range(NG):
            ps = pp_qkv.tile([P, C], f32, tag="big")
            for cc in range(4):
                nc.tensor.matmul(
                    ps[:], hT_sb[:, cc, g * P:(g + 1) * P],
                    wqkv_sb[:, cc, 2 * C:3 * C],
                    start=(cc == 0), stop=(cc == 3))
            nc.any.tensor_tensor(v_sb[:, g], ps[:], bvbc_sb[:], ALU.add)
        return qk_sb, v_sb

    def attn_cc(cc, qk_sb, v_sb, oT_sb, pe_filler):
        # QK^T for 4 heads, one PSUM bank per head-tile i (concurrent
        # row-packed matmuls must drain into distinct banks); only the
        # first 256 fp32 of each bank are used.
        pss = pp_s.tile([P, 4, TB], f32, tag="pss")
        psv = pss[:, :, 0:NG * 64].rearrange("p i (g q) -> p i g q", g=NG)
        for w in range(8):
            pairi, wcl = divmod(w, 2)
            for i in range(4):
                out_sl = (psv[0:64, i, pairi] if wcl == 0
                          else psv[64:128, i, pairi])
                nc.tensor.matmul(
                    out_sl,
                    qk_sb[32 * i:32 * (i + 1), 1, cc, w * 64:(w + 1) * 64],
                    qk_sb[32 * i:32 * (i + 1), 0, cc, w * 64:(w + 1) * 64],
                    start=True, stop=True,
                    tile_position=(32 * i, 0 if wcl == 0 else 64))
        nc.vector.tensor_tensor(
            psv[:], psv[:],
            biasT_sb[:, 4 * cc:4 * cc + 4, None, :].to_broadcast(
                (P, 4, NG, 64)),
            ALU.add)
        nc.scalar.activation(scores_sb[0:64, 4 * cc:4 * cc + 4, :, 0, :],
                             psv[0:64], AF.Exp)
        nc.scalar.activation(scores_sb[64:128, 4 * cc:4 * cc + 4, :, 1, :],
                             psv[64:128], AF.Exp)
        # PE filler work (prev band's mlp1) lands here in the PE stream, so
        # the engine computes through the bias+exp chain instead of
        # stalling on the pd/AV matmuls that need the fresh scores
        pe_filler()
        # softmax denominators: col-packed ones-matmuls + fast reciprocal
        pd = pp_qkv.tile([P, NG, P], f32, tag="big")
        for pair in range(NG):
            for j in range(4):
                nc.tensor.matmul(
                    pd[32 * j:32 * (j + 1), pair], ones_sb[:],
                    scores_sb[:, 4 * cc + j, pair],
                    start=True, stop=True, tile_position=(0, 32 * j))
        rd = rdpool.tile([P, NG, P], f32, tag="rd")
        nc.vector.reciprocal_approx_fast(rd[:], pd[:])
        # AV col-packed 4 heads into one PSUM bank; normalize on the
        # PSUM->SBUF evacuation multiply
        po = pp_qkv.tile([P, NG, P], f32, tag="big")
        for pair in range(NG):
            for j in range(4):
                nc.tensor.matmul(
                    po[32 * j:32 * (j + 1), pair],
                    v_sb[:, pair, (4 * cc + j) * HD:(4 * cc + j + 1) * HD],
                    scores_sb[:, 4 * cc + j, pair],
                    start=True, stop=True, tile_position=(0, 32 * j))
        nc.vector.tensor_tensor(oT_sb[:, cc], po[:], rd[:], ALU.mult)

    def mlp1_chunk(m1_sb, h2T_sb, fcs):
        for fc in fcs:
            ps = pp_mlp.tile([P, TB], f32, tag="mlp")
            for cc in range(4):
                nc.tensor.matmul(
                    ps[:], w1_sb[:, cc, fc], h2T_sb[:, cc],
                    start=(cc == 0), stop=(cc == 3))
            nc.scalar.activation(m1_sb[:, fc], ps[:], AF.Relu,
                                 bias=bm1c_sb[:, fc:fc + 1])

    def mlp2_g(prev, g):
        m1_sb, y_sb, pband = prev
        ps = pp_mlp.tile([P, C], f32, tag="mlp")
        for fc in range(16):
            nc.tensor.matmul(
                ps[:], m1_sb[:, fc, g * P:(g + 1) * P], w2_sb[:, fc],
                start=(fc == 0), stop=(fc == 15))
        o_sb = outpool.tile([P, C], f32, tag="out")
        nc.any.tensor_tensor(o_sb[:], ps[:], y_sb[:, g], ALU.add)
        nc.any.tensor_tensor(o_sb[:], o_sb[:], bm2bc_sb[:], ALU.add)
        for wcl in range(2):
            nc.gpsimd.dma_start(out_r[pband, g, wcl],
                                o_sb[wcl * 64:(wcl + 1) * 64])

    # ---- prologue: band 0 LN1 + qkv ----
    x_sb, mv1 = load_stats(0)
    hT_sb = ln_apply_transpose(x_sb, ln_rstd(mv1, "1"), "hT")
    qk_sb, v_sb = emit_qkv(hT_sb)
    prev = None  # (m1, y, band) of the band whose MLP is still pending

    for band in range(NB):
        # next band's x loads + LN stats go first: the gpsimd ring and DVE
        # have slack at the start of the attention block
        if band + 1 < NB:
            nxt_x, nxt_mv = load_stats(band + 1)

        # ---- attention(band), interleaving mlp1(band-1) as PE filler ----
        oT_sb = opool.tile([P, 4, TB], bf16, tag="oT")
        if prev is not None:
            m1_sb = m1pool.tile([P, 16, TB], bf16, tag="m1")
            h2T_prev = prev_h2T
            fillers = [lambda: mlp1_chunk(m1_sb, h2T_prev, range(0, 4)),
                       lambda: mlp1_chunk(m1_sb, h2T_prev, range(4, 8)),
                       lambda: mlp1_chunk(m1_sb, h2T_prev, range(8, 12)),
                       lambda: mlp1_chunk(m1_sb, h2T_prev, range(12, 16))]
        else:
            fillers = [lambda: None] * 4
        attn_cc(0, qk_sb, v_sb, oT_sb, fillers[0])
        attn_cc(1, qk_sb, v_sb, oT_sb, fillers[1])
        if band + 1 < NB:
            nxt_ab = ln_rstd(nxt_mv, "1")
        attn_cc(2, qk_sb, v_sb, oT_sb, fillers[2])
        attn_cc(3, qk_sb, v_sb, oT_sb, fillers[3])
        # LN1(band+1) applies + transposes: ACT/DVE/sync are free once the
        # last exps are issued; hT(band+1) is ready by proj end
        if band + 1 < NB:
            nxt_hT = ln_apply_transpose(nxt_x, nxt_ab, "hT")

        # ---- proj(band) + residual + LN2 stats, interleaving
        # mlp2(band-1) + stores ----
        st2 = spool.tile([P, NG, 6], f32, tag="st2", name="st2")
        mv2 = spool.tile([P, NG, 2], f32, tag="mv2", name="mv2")
        for g in range(NG):
            ps = pp_mlp.tile([P, C], f32, tag="mlp")
            for cc in range(4):
                nc.tensor.matmul(
                    ps[:], oT_sb[:, cc, g * P:(g + 1) * P],
                    wproj_sb[:, cc],
                    start=(cc == 0), stop=(cc == 3))
            nc.any.tensor_tensor(x_sb[:, g], ps[:], x_sb[:, g], ALU.add)
            nc.any.tensor_tensor(x_sb[:, g], x_sb[:, g], bpbc_sb[:], ALU.add)
            nc.vector.bn_stats(out=st2[:, g], in_=x_sb[:, g])
            nc.vector.bn_aggr(out=mv2[:, g], in_=st2[:, g])
            if prev is not None:
                mlp2_g((m1_sb, prev[1], prev[2]), g)
        y_sb = x_sb

        # ---- qkv(band+1) keeps the PE stream unbroken while the LN2
        # chain below runs on DVE/ACT/sync ----
        if band + 1 < NB:
            qk_sb, v_sb = emit_qkv(nxt_hT)
        h2T_sb = ln_apply_transpose(y_sb, ln_rstd(mv2, "2"), "h2T")

        prev = (None, y_sb, band)
        prev_h2T = h2T_sb
        if band + 1 < NB:
            x_sb = nxt_x

    # ---- epilogue: last band's MLP ----
    m1_sb = m1pool.tile([P, 16, TB], bf16, tag="m1")
    mlp1_chunk(m1_sb, prev_h2T, range(16))
    for g in range(NG):
        mlp2_g((m1_sb, prev[1], prev[2]), g)


@functools.lru_cache(maxsize=2)
def _build():
    from contextlib import ExitStack
    import concourse.mybir as mybir
    import concourse.tile as tile
    from concourse import bacc

    dt = mybir.dt
    nc = bacc.Bacc("TRN2", target_bir_lowering=False, debug=False,
                   num_devices=N_CORES)
    aps = {}
    specs = [
        ("x", [H, W, C], dt.float32),
        ("wqkv", [C, 3 * C], dt.bfloat16),
        ("wproj", [C, C], dt.bfloat16),
        ("w1", [C, 4 * C], dt.bfloat16),
        ("w2", [4 * C, C], dt.bfloat16),
        ("bqkc", [P, 8], dt.float32),
        ("bm1c", [P, 16], dt.float32),
        ("bvbc", [P, C], dt.bfloat16),
        ("bpbc", [P, C], dt.bfloat16),
        ("bm2bc", [P, C], dt.bfloat16),
        ("biasT", [P, NH, 64], dt.bfloat16),
    ]
    for name, shape, dtype in specs:
        aps[name] = nc.dram_tensor(name, shape, dtype,
                                   kind="ExternalInput").ap()
    aps["out"] = nc.dram_tensor("out", [H, W, C], dt.float32,
                                kind="ExternalOutput").ap()
    with tile.TileContext(nc) as tc:
        with ExitStack() as ctx:
            _emit(nc, tc, ctx, aps)
    nc.compile()
    return nc


def _prepare_in_maps(x, g1, b1, wqkv, bqkv, wproj, bproj, rel_bias, g2, b2,
                     w1, bm1, w2, bm2):
    x = np.asarray(x, np.float32)
    f = lambda a: np.ascontiguousarray(np.asarray(a, np.float32))
    g1, b1, wqkv, bqkv = f(g1), f(b1), f(wqkv), f(bqkv)
    wproj, bproj, rel_bias = f(wproj), f(bproj), f(rel_bias)
    g2, b2, w1, bm1, w2, bm2 = f(g2), f(b2), f(w1), f(bm1), f(w2), f(bm2)

    # fold LN1 affine into wqkv/bqkv; fold attention scale into q weights
    wqkv_f = g1[:, None] * wqkv
    bqkv_f = b1 @ wqkv + bqkv
    sc = HD ** -0.5
    wqkv_f[:, :C] *= sc
    bqkv_f[:C] *= sc
    # fold LN2 affine into w1/bm1
    w1_f = g2[:, None] * w1
    bm1_f = b2 @ w1 + bm1

    bqkc = np.ascontiguousarray(bqkv_f[:2 * C].reshape(8, P).T)   # [128, 8]
    bm1c = np.ascontiguousarray(bm1_f.reshape(16, P).T)           # [128, 16]
    import ml_dtypes
    bfarr = lambda a: np.ascontiguousarray(a).astype(ml_dtypes.bfloat16)
    bvbc = bfarr(np.broadcast_to(bqkv_f[2 * C:], (P, C)))
    bpbc = bfarr(np.broadcast_to(bproj, (P, C)))
    bm2bc = bfarr(np.broadcast_to(bm2, (P, C)))

    idx = _rel_pos_index()                              # [64(n), 64(m)]
    bias_nm = rel_bias[idx, :]                          # [n, m, NH]
    biasT_h = bias_nm.transpose(2, 1, 0)                # [NH, m, n]
    biasT = np.concatenate([biasT_h, biasT_h], axis=1)  # [NH, 128, 64]
    biasT = bfarr(biasT.transpose(1, 0, 2))             # [128, NH, 64]

    wqkv_b, wproj_b, w1_b, w2_b = (bfarr(wqkv_f), bfarr(wproj),
                                   bfarr(w1_f), bfarr(w2))
    shared = dict(wqkv=wqkv_b, wproj=wproj_b, w1=w1_b, w2=w2_b,
                  bqkc=bqkc, bm1c=bm1c, bvbc=bvbc, bpbc=bpbc, bm2bc=bm2bc,
                  biasT=biasT)
    return [dict(x=np.ascontiguousarray(x[c]), **shared)
            for c in range(N_CORES)]


def kernel(**inputs):
    from concourse.bass_utils import run_bass_kernel_spmd

    in_maps = _prepare_in_maps(**inputs)
    nc = _build()
    res = run_bass_kernel_spmd(nc, in_maps, core_ids=list(range(N_CORES)))
    return np.stack([res.results[c]["out"] for c in range(N_CORES)], axis=0)
